# revision 1
# baseline (speedup 1.0000x reference)
"""Trainium2 Bass kernel for nn_Correspondence (retrieval_knn).

Pipeline per clip (B=4 clips, snip=8 frames of 28x28, C=256):
  xs = [C, THW=6272] per clip; corr = cosine similarity over channels;
  per column j: top-5 rows t (same-frame block excluded) -> gather xs cols,
  max over the 5 -> y; global BatchNorm (training stats) + relu -> 1x1 conv
  -> + identity.

Sharding: 8 cores = 4 clips x 2 column-halves. Each core gets its clip's
xs ROTATED by half the frames so its own j-range is local columns [0,3136)
— the same SPMD program runs on all cores. Same-frame masking is handled
by *never computing* the own-frame columns (frame-rotated chunk order).

Precision: the correlation matmul runs as main term in float32r (12-bit
mantissa, full PE rate) plus bf16 cross-correction terms (bx^T be + be^T bx
where be = bf16(xn - f32r(xn))) — reproduces fp32 top-5 ordering exactly
(verified 0/25088 column set flips vs fp64 on the real data distribution).
Gather/BN run in exact fp32; the 1x1 conv uses single f32r.
"""
import sys, os
import numpy as np

for _p in ("/opt/trn_rl_repo", "/root/.axon_site/_ro/trn_rl_repo"):
    if os.path.isdir(_p) and _p not in sys.path:
        sys.path.insert(0, _p)
        break

import ml_dtypes

# ---------------- problem constants (hardcoded) ----------------
C = 256          # channels
SNIP = 8         # frames per clip
F = 784          # 28*28
T = SNIP * F     # 6272 columns per clip
J = T // 2       # 3136 columns handled per core
JT = 112         # j-tile rows (one PE M-tile; 112*7 = 784 -> tiles never span frames)
NT = J // JT     # 28 j-tiles
TS = 7 * F       # 5488 searched columns per j-tile (own frame excluded)
HALF = TS // 2   # 2744
QUART = TS // 4  # 1372
KTOP = 5
NCORES = 8
NTOT = 32 * F    # batchnorm count = BS*H*W = 25088
CW = 392         # matmul chunk width (one PSUM bank)
GAT = JT * 8     # 896 gather indices per j-tile (top-5 + 3 duplicated slots)

_CACHE = {}


def _round_f32r(x):
    """Round-to-nearest-even to f32r (low 12 mantissa bits zeroed)."""
    b = np.ascontiguousarray(x, np.float32).view(np.uint32)
    low = b & np.uint32(0xFFF)
    add = (low > 0x800) | ((low == 0x800) & (((b >> 12) & 1) == 1))
    b = (b & ~np.uint32(0xFFF)) + (add.astype(np.uint32) << 12)
    return b.view(np.float32)


def _build(num_cores, dbg=False):
    import concourse.bass as bass
    import concourse.mybir as mybir
    import concourse.tile as tile
    from concourse import bacc
    from concourse.masks import make_identity

    fp32 = mybir.dt.float32
    f32r = mybir.dt.float32r
    bf16 = mybir.dt.bfloat16
    i16 = mybir.dt.int16
    u16 = mybir.dt.uint16
    Alu = mybir.AluOpType
    Act = mybir.ActivationFunctionType
    Ax = mybir.AxisListType

    nc = bacc.Bacc("TRN2", target_bir_lowering=False, debug=False,
                   num_devices=num_cores)

    xs_d = nc.declare_dram_parameter("xs", [C, T], fp32, isOutput=False)
    r_d = nc.declare_dram_parameter("xr", [C, T], f32r, isOutput=False)
    bx_d = nc.declare_dram_parameter("bx", [C, T], bf16, isOutput=False)
    be_d = nc.declare_dram_parameter("be", [C, T], bf16, isOutput=False)
    wt_d = nc.declare_dram_parameter("wt", [C, C], f32r, isOutput=False)
    gam_d = nc.declare_dram_parameter("gam", [C], fp32, isOutput=False)
    bet_d = nc.declare_dram_parameter("bet", [C], fp32, isOutput=False)
    cb_d = nc.declare_dram_parameter("cb", [C], fp32, isOutput=False)
    out_d = nc.declare_dram_parameter("out", [4, C, F], fp32, isOutput=True)
    if dbg:
        dbg_sA = nc.declare_dram_parameter("dbg_sA", [JT, HALF], fp32, isOutput=True)
        dbg_sB = nc.declare_dram_parameter("dbg_sB", [JT, HALF], fp32, isOutput=True)
        dbg_t8 = nc.declare_dram_parameter("dbg_t8", [JT, 8], fp32, isOutput=True)
        dbg_ia = nc.declare_dram_parameter("dbg_ia", [JT, 4, 8], mybir.dt.uint16, isOutput=True)
        dbg_fin = nc.declare_dram_parameter("dbg_fin", [JT, 8], fp32, isOutput=True)
        dbg_w16 = nc.declare_dram_parameter("dbg_w16", [128, GAT // 16], i16, isOutput=True)
        dbg_gat = nc.declare_dram_parameter("dbg_gat", [128, GAT], fp32, isOutput=True)
        dbg_yt = nc.declare_dram_parameter("dbg_yt", [128, JT], fp32, isOutput=True)

    with tile.TileContext(nc) as tc:
        with tc.tile_pool(name="singles", bufs=1) as sg, \
             tc.tile_pool(name="dram", bufs=1, space="DRAM") as dp:
            # ---- persistent inputs in SBUF
            xs0 = sg.tile([128, T], fp32)
            xs1 = sg.tile([128, T], fp32)
            r0 = sg.tile([128, T], f32r)
            r1 = sg.tile([128, T], f32r)
            bx0 = sg.tile([128, T], bf16)
            bx1 = sg.tile([128, T], bf16)
            be0 = sg.tile([128, T], bf16)
            be1 = sg.tile([128, T], bf16)
            wt0 = sg.tile([128, C], f32r)
            wt1 = sg.tile([128, C], f32r)
            gam = sg.tile([128, 2], fp32)
            bet = sg.tile([128, 2], fp32)
            cbv = sg.tile([128, 2], fp32)
            ident = sg.tile([128, 128], fp32)
            bases = sg.tile([112, 4, 8], fp32)
            stats = sg.tile([128, 4, NT], fp32)
            astat = sg.tile([128, 4], fp32)
            scales = sg.tile([128, 2], fp32)
            shifts = sg.tile([128, 2], fp32)
            scr = sg.tile([128, JT], fp32)

            y_dram = dp.tile([2, 128, J], fp32)
            cc_in = dp.tile([128, 4], fp32)
            cc_out = dp.tile([128, 4], fp32, addr_space="Shared")

            nc.sync.dma_start(out=r0, in_=r_d[0:128, :])
            nc.sync.dma_start(out=r1, in_=r_d[128:256, :])
            nc.sync.dma_start(out=bx0, in_=bx_d[0:128, :])
            nc.sync.dma_start(out=bx1, in_=bx_d[128:256, :])
            nc.sync.dma_start(out=be0, in_=be_d[0:128, :])
            nc.sync.dma_start(out=be1, in_=be_d[128:256, :])
            nc.sync.dma_start(out=xs0, in_=xs_d[0:128, :])
            nc.sync.dma_start(out=xs1, in_=xs_d[128:256, :])
            nc.sync.dma_start(out=wt0, in_=wt_d[0:128, :])
            nc.sync.dma_start(out=wt1, in_=wt_d[128:256, :])
            nc.sync.dma_start(out=gam[:, 0:1], in_=gam_d[0:128])
            nc.sync.dma_start(out=gam[:, 1:2], in_=gam_d[128:256])
            nc.sync.dma_start(out=bet[:, 0:1], in_=bet_d[0:128])
            nc.sync.dma_start(out=bet[:, 1:2], in_=bet_d[128:256])
            nc.sync.dma_start(out=cbv[:, 0:1], in_=cb_d[0:128])
            nc.sync.dma_start(out=cbv[:, 1:2], in_=cb_d[128:256])

            make_identity(nc, ident)
            for k in range(4):
                nc.vector.memset(bases[:, k, :], float(k * QUART))

            xs_t = (xs0, xs1)

            with tc.tile_pool(name="spool", bufs=1) as sp, \
                 tc.tile_pool(name="work", bufs=2) as wk, \
                 tc.tile_pool(name="gatp", bufs=1) as gp, \
                 tc.tile_pool(name="pp", bufs=6, space="PSUM") as pp, \
                 tc.tile_pool(name="pt", bufs=2, space="PSUM") as pt:

                for jt in range(NT):
                    f = jt // 7                      # local frame of this j-tile
                    j0 = jt * JT
                    sA = sp.tile([JT, HALF], fp32, tag="sA")
                    sB = sp.tile([JT, HALF], fp32, tag="sB")

                    # ---- correlation matmuls, frame-rotated column order
                    # chunks of 392 (= half frame): one PSUM bank each
                    for ci in range(14):
                        g = (f + 1 + ci // 2) % SNIP  # source frame for chunk
                        gc = g * F + (ci % 2) * CW
                        ps = pp.tile([JT, CW], fp32, tag="ps")
                        nc.tensor.matmul(ps, r0[:, j0:j0 + JT],
                                         r0[:, gc:gc + CW], start=True, stop=False)
                        nc.tensor.matmul(ps, r1[:, j0:j0 + JT],
                                         r1[:, gc:gc + CW], start=False, stop=False)
                        nc.tensor.matmul(ps, bx0[:, j0:j0 + JT],
                                         be0[:, gc:gc + CW], start=False, stop=False)
                        nc.tensor.matmul(ps, bx1[:, j0:j0 + JT],
                                         be1[:, gc:gc + CW], start=False, stop=False)
                        nc.tensor.matmul(ps, be0[:, j0:j0 + JT],
                                         bx0[:, gc:gc + CW], start=False, stop=False)
                        nc.tensor.matmul(ps, be1[:, j0:j0 + JT],
                                         bx1[:, gc:gc + CW], start=False, stop=True)
                        # drain PSUM -> s (ACT engine); 2744 = 7*392 exactly
                        if ci < 7:
                            nc.scalar.copy(sA[:, ci * CW:(ci + 1) * CW], ps[:])
                        else:
                            nc.scalar.copy(sB[:, (ci - 7) * CW:(ci - 6) * CW],
                                           ps[:])

                    if dbg and jt == 0:
                        nc.sync.dma_start(out=dbg_sA[:], in_=sA)
                        nc.sync.dma_start(out=dbg_sB[:], in_=sB)
                    # ---- top-8 values + indices (exact fp32)
                    t8ab = wk.tile([JT, 16], fp32, tag="t8ab")
                    t8 = wk.tile([JT, 8], fp32, tag="t8")
                    iall = wk.tile([JT, 4, 8], u16, tag="iall")
                    nc.vector.max(out=t8ab[:, 0:8], in_=sA)
                    nc.vector.max(out=t8ab[:, 8:16], in_=sB)
                    nc.vector.max(out=t8, in_=t8ab)
                    nc.vector.max_index(out=iall[:, 0, :], in_max=t8,
                                        in_values=sA[:, 0:QUART])
                    nc.vector.max_index(out=iall[:, 1, :], in_max=t8,
                                        in_values=sA[:, QUART:HALF])
                    nc.vector.max_index(out=iall[:, 2, :], in_max=t8,
                                        in_values=sB[:, 0:QUART])
                    nc.vector.max_index(out=iall[:, 3, :], in_max=t8,
                                        in_values=sB[:, QUART:HALF])

                    # ---- combine quarters -> global column index
                    fall = wk.tile([JT, 4, 8], fp32, tag="fall")
                    m01 = wk.tile([JT, 8], fp32, tag="m01")
                    m23 = wk.tile([JT, 8], fp32, tag="m23")
                    gmin = wk.tile([JT, 8], fp32, tag="gmin")
                    msk = wk.tile([JT, 8], fp32, tag="msk")
                    fin = wk.tile([JT, 8], fp32, tag="fin")
                    fdup = wk.tile([JT, 8], fp32, tag="fdup")
                    nc.vector.tensor_copy(fall, iall)          # u16 -> fp32 (65535 if absent)
                    nc.vector.tensor_add(fall, fall, bases)
                    nc.vector.tensor_tensor(out=m01, in0=fall[:, 0, :],
                                            in1=fall[:, 1, :], op=Alu.min)
                    nc.vector.tensor_tensor(out=m23, in0=fall[:, 2, :],
                                            in1=fall[:, 3, :], op=Alu.min)
                    nc.vector.tensor_tensor(out=gmin, in0=m01, in1=m23, op=Alu.min)
                    # searched col c -> clip col t = ((f+1)*784 + c) mod 6272
                    nc.vector.tensor_scalar_add(gmin, gmin, float((f + 1) * F))
                    nc.vector.tensor_scalar(out=msk, in0=gmin, scalar1=float(T),
                                            scalar2=None, op0=Alu.is_ge)
                    nc.vector.scalar_tensor_tensor(out=fin, in0=msk,
                                                   scalar=float(-T), in1=gmin,
                                                   op0=Alu.mult, op1=Alu.add)
                    nc.vector.tensor_copy(fdup[:, 0:5], fin[:, 0:5])
                    nc.vector.tensor_copy(fdup[:, 5:8],
                                          fin[:, 0:1].to_broadcast([JT, 3]))

                    if dbg and jt == 0:
                        nc.sync.dma_start(out=dbg_t8[:], in_=t8)
                        nc.sync.dma_start(out=dbg_ia[:], in_=iall)
                        nc.sync.dma_start(out=dbg_fin[:], in_=fin)
                    # ---- wrapped int16 index list for ap_gather
                    trp = pt.tile([8, JT], fp32, tag="tr")
                    nc.tensor.transpose(trp, fdup, ident[0:JT, 0:JT])
                    trs = wk.tile([8, JT], i16, tag="trs")
                    nc.vector.tensor_copy(trs, trp)
                    w16 = wk.tile([128, GAT // 16], i16, tag="w16")
                    trv = trs.rearrange("p (m two) -> p m two", two=2)
                    nc.sync.dma_start(out=w16[0:8, :], in_=trv[:, :, 0])
                    nc.sync.dma_start(out=w16[8:16, :], in_=trv[:, :, 1])
                    nc.sync.dma_start(out=w16[16:32, :], in_=w16[0:16, :])
                    nc.sync.dma_start(out=w16[32:64, :], in_=w16[0:32, :])
                    nc.sync.dma_start(out=w16[64:128, :], in_=w16[0:64, :])
                    if dbg and jt == 0:
                        nc.sync.dma_start(out=dbg_w16[:], in_=w16)

                    # ---- gather + max over the 5 picks (+3 dups)
                    for c in range(2):
                        gat = gp.tile([128, GAT], fp32, tag=f"gat{c}")
                        nc.gpsimd.ap_gather(out_ap=gat[:], in_ap=xs_t[c][:],
                                            idxs_ap=w16[:], channels=128,
                                            num_elems=T, d=1, num_idxs=GAT)
                        yt = gp.tile([128, JT], fp32, tag=f"yt{c}")
                        gv = gat.rearrange("p (j k) -> p j k", k=8)
                        nc.vector.tensor_reduce(out=yt, in_=gv, axis=Ax.X,
                                                op=Alu.max)
                        # batchnorm partial sums (ACT accumulator)
                        nc.scalar.activation(scr, yt, Act.Identity,
                                             accum_out=stats[:, 2 * c, jt:jt + 1])
                        nc.scalar.activation(scr, yt, Act.Square,
                                             accum_out=stats[:, 2 * c + 1, jt:jt + 1])
                        nc.sync.dma_start(out=y_dram[c, :, j0:j0 + JT], in_=yt)
                        if dbg and jt == 0 and c == 0:
                            nc.sync.dma_start(out=dbg_gat[:], in_=gat)
                            nc.sync.dma_start(out=dbg_yt[:], in_=yt)

            # ---- global batchnorm stats (allreduce over the 8 cores)
            nc.vector.tensor_reduce(out=astat, in_=stats, axis=Ax.X, op=Alu.add)
            nc.sync.dma_start(out=cc_in[:], in_=astat)
            nc.gpsimd.collective_compute(
                "AllReduce", Alu.add,
                replica_groups=[list(range(num_cores))],
                ins=[cc_in[:].opt()], outs=[cc_out[:].opt()])
            nc.sync.dma_start(out=astat, in_=cc_out[:])

            with tc.tile_pool(name="bnw", bufs=1) as bw:
                mean = bw.tile([128, 2], fp32)
                ex2 = bw.tile([128, 2], fp32)
                var = bw.tile([128, 2], fp32)
                std = bw.tile([128, 2], fp32)
                rstd = bw.tile([128, 2], fp32)
                vv = astat.rearrange("p (c m) -> p c m", m=2)
                nc.vector.tensor_scalar_mul(mean, vv[:, :, 0], 1.0 / NTOT)
                nc.vector.tensor_scalar_mul(ex2, vv[:, :, 1], 1.0 / NTOT)
                nc.vector.tensor_tensor(out=var, in0=mean, in1=mean, op=Alu.mult)
                nc.vector.tensor_sub(var, ex2, var)
                nc.vector.tensor_scalar_add(var, var, 1e-5)
                nc.scalar.sqrt(std, var)
                nc.vector.reciprocal(rstd, std)
                nc.vector.tensor_tensor(out=scales, in0=gam, in1=rstd, op=Alu.mult)
                nc.vector.tensor_tensor(out=shifts, in0=mean, in1=scales,
                                        op=Alu.mult)
                nc.vector.tensor_sub(shifts, bet, shifts)

            # ---- BN apply + relu + 1x1 conv + identity + store
            with tc.tile_pool(name="zp", bufs=2) as zp, \
                 tc.tile_pool(name="cp", bufs=2, space="PSUM") as cp:
                for ci in range(8):
                    c0 = ci * CW
                    yi0 = zp.tile([128, CW], fp32, tag="yi0")
                    yi1 = zp.tile([128, CW], fp32, tag="yi1")
                    nc.sync.dma_start(out=yi0, in_=y_dram[0, :, c0:c0 + CW])
                    nc.sync.dma_start(out=yi1, in_=y_dram[1, :, c0:c0 + CW])
                    z0 = zp.tile([128, CW], f32r, tag="z0")
                    z1 = zp.tile([128, CW], f32r, tag="z1")
                    nc.scalar.activation(z0, yi0, Act.Relu,
                                         bias=shifts[:, 0:1], scale=scales[:, 0:1])
                    nc.scalar.activation(z1, yi1, Act.Relu,
                                         bias=shifts[:, 1:2], scale=scales[:, 1:2])
                    fr, fc = divmod(ci, 2)
                    for ot in range(2):
                        o0 = ot * 128
                        cps = cp.tile([128, CW], fp32, tag="cps")
                        nc.tensor.matmul(cps, wt0[:, o0:o0 + 128], z0[:],
                                         start=True, stop=False)
                        nc.tensor.matmul(cps, wt1[:, o0:o0 + 128], z1[:],
                                         start=False, stop=True)
                        osb = zp.tile([128, CW], fp32, tag=f"osb{ot}")
                        nc.vector.scalar_tensor_tensor(
                            out=osb, in0=cps, scalar=cbv[:, ot:ot + 1],
                            in1=xs_t[ot][:, c0:c0 + CW], op0=Alu.add, op1=Alu.add)
                        nc.sync.dma_start(
                            out=out_d[fr, o0:o0 + 128, fc * CW:(fc + 1) * CW],
                            in_=osb)

    nc.finalize()
    return nc


def _get_nc(num_cores):
    if num_cores not in _CACHE:
        _CACHE[num_cores] = _build(num_cores)
    return _CACHE[num_cores]


def _prep_core_inputs(x, conv_w, gamma, beta, conv_b):
    """Build the 8 per-core input dicts from the full problem inputs."""
    xs_all = np.ascontiguousarray(
        x.reshape(4, SNIP, C, F).transpose(0, 2, 1, 3).reshape(4, C, T))
    wt = _round_f32r(np.ascontiguousarray(conv_w.T))
    bf = ml_dtypes.bfloat16
    maps = []
    for k in range(NCORES):
        b, h = divmod(k, 2)
        xs = xs_all[b]
        if h:
            xs = np.ascontiguousarray(
                np.concatenate((xs[:, J:], xs[:, :J]), axis=1))
        nrm = np.sqrt((xs * xs).sum(0, dtype=np.float32))
        xn = xs * (1.0 / nrm)[None, :].astype(np.float32)
        r = _round_f32r(xn)
        maps.append({
            "xs": xs,
            "xr": r,
            "bx": xn.astype(bf),
            "be": (xn - r).astype(bf),
            "wt": wt,
            "gam": np.ascontiguousarray(gamma, np.float32),
            "bet": np.ascontiguousarray(beta, np.float32),
            "cb": np.ascontiguousarray(conv_b, np.float32),
        })
    return maps


def kernel(x, gamma, beta, conv_w, conv_b, snip):
    assert int(snip) == SNIP and x.shape == (32, C, 28, 28)
    from concourse.bass_utils import run_bass_kernel_spmd

    x = np.ascontiguousarray(x, np.float32)
    maps = _prep_core_inputs(x, np.asarray(conv_w, np.float32),
                             np.asarray(gamma, np.float32),
                             np.asarray(beta, np.float32),
                             np.asarray(conv_b, np.float32))
    nc = _get_nc(NCORES)
    res = run_bass_kernel_spmd(nc, maps, list(range(NCORES))).results
    out = np.empty((32, C, F), np.float32)
    for k in range(NCORES):
        out[4 * k:4 * k + 4] = res[k]["out"]
    return out.reshape(32, C, 28, 28)



# revision 2
# speedup vs baseline: 1.0991x; 1.0991x over previous
"""Trainium2 Bass kernel for nn_Correspondence (retrieval_knn).

Pipeline per clip (B=4 clips, snip=8 frames of 28x28, C=256):
  xs = [C, THW=6272] per clip; corr = cosine similarity over channels;
  per column j: top-5 rows t (same-frame block excluded) -> gather xs cols,
  max over the 5 -> y; global BatchNorm (training stats) + relu -> 1x1 conv
  -> + identity.

Sharding: 8 cores = 4 clips x 2 column-halves. Each core gets its clip's
xs ROTATED by half the frames so its own j-range is local columns [0,3136)
— the same SPMD program runs on all cores. Same-frame masking is handled
by *never computing* the own-frame columns (frame-rotated chunk order).

Precision: the correlation matmul runs in float32r only (12-bit mantissa,
fp32 accumulate). On the actual seed-0 data this flips the top-5 set on
~43/25088 columns vs exact fp32, giving end-to-end rel err ~5e-3 — well
under the 2e-2 gate. Gather/BN run in exact fp32; the 1x1 conv uses f32r.

Schedule: per j-tile (112 cols, 28 tiles) the 14 chunk matmuls accumulate
in PSUM and drain (ACT) into a double-buffered [112,5488] score tile; DVE
does one max8 + one max_index over the full 5488, index-wrap math, then a
PE transpose + small DMA chain builds the wrapped int16 gather list; the
GpSimd ap_gather + DVE max-reduce write y straight into SBUF. Double
buffering overlaps each tile's top-k/gather tail with the next tile's
matmul phase. BN stats accumulate per-tile on ACT; one AllReduce at the
end, then BN+relu+1x1 conv straight out of SBUF.
"""
import sys, os
import numpy as np

for _p in ("/opt/trn_rl_repo", "/root/.axon_site/_ro/trn_rl_repo"):
    if os.path.isdir(_p) and _p not in sys.path:
        sys.path.insert(0, _p)
        break

# ---------------- problem constants (hardcoded) ----------------
C = 256          # channels
SNIP = 8         # frames per clip
F = 784          # 28*28
T = SNIP * F     # 6272 columns per clip
J = T // 2       # 3136 columns handled per core
JT = 112         # j-tile rows (one PE M-tile; 112*7 = 784 -> tiles never span frames)
NT = J // JT     # 28 j-tiles
TS = 7 * F       # 5488 searched columns per j-tile (own frame excluded)
KTOP = 5
NCORES = 8
NTOT = 32 * F    # batchnorm count = BS*H*W = 25088
CW = 392         # matmul chunk width (one PSUM bank)
GAT = JT * 8     # 896 gather indices per j-tile (top-5 + 3 duplicated slots)

_CACHE = {}


def _round_f32r(x):
    """Round-to-nearest-even to f32r (low 12 mantissa bits zeroed)."""
    b = np.ascontiguousarray(x, np.float32).view(np.uint32)
    low = b & np.uint32(0xFFF)
    add = (low > 0x800) | ((low == 0x800) & (((b >> 12) & 1) == 1))
    b = (b & ~np.uint32(0xFFF)) + (add.astype(np.uint32) << 12)
    return b.view(np.float32)


def _build(num_cores, dbg=False):
    import concourse.bass as bass
    import concourse.mybir as mybir
    import concourse.tile as tile
    from concourse import bacc
    from concourse.masks import make_identity

    fp32 = mybir.dt.float32
    f32r = mybir.dt.float32r
    i16 = mybir.dt.int16
    u16 = mybir.dt.uint16
    Alu = mybir.AluOpType
    Act = mybir.ActivationFunctionType
    Ax = mybir.AxisListType

    nc = bacc.Bacc("TRN2", target_bir_lowering=False, debug=False,
                   num_devices=num_cores)

    xs_d = nc.declare_dram_parameter("xs", [C, T], fp32, isOutput=False)
    r_d = nc.declare_dram_parameter("xr", [C, T], f32r, isOutput=False)
    wt_d = nc.declare_dram_parameter("wt", [C, C], f32r, isOutput=False)
    gam_d = nc.declare_dram_parameter("gam", [C], fp32, isOutput=False)
    bet_d = nc.declare_dram_parameter("bet", [C], fp32, isOutput=False)
    cb_d = nc.declare_dram_parameter("cb", [C], fp32, isOutput=False)
    out_d = nc.declare_dram_parameter("out", [4, C, F], fp32, isOutput=True)
    if dbg:
        dbg_s = nc.declare_dram_parameter("dbg_s", [JT, TS], fp32, isOutput=True)
        dbg_t8 = nc.declare_dram_parameter("dbg_t8", [JT, 8], fp32, isOutput=True)
        dbg_i8 = nc.declare_dram_parameter("dbg_i8", [JT, 8], mybir.dt.uint16, isOutput=True)
        dbg_fin = nc.declare_dram_parameter("dbg_fin", [JT, 8], fp32, isOutput=True)
        dbg_w16 = nc.declare_dram_parameter("dbg_w16", [128, GAT // 16], i16, isOutput=True)
        dbg_yt = nc.declare_dram_parameter("dbg_yt", [128, JT], fp32, isOutput=True)

    with tile.TileContext(nc) as tc:
        with tc.tile_pool(name="singles", bufs=1) as sg, \
             tc.tile_pool(name="dram", bufs=1, space="DRAM") as dp:
            # ---- persistent inputs in SBUF
            r0 = sg.tile([128, T], f32r)
            r1 = sg.tile([128, T], f32r)
            xs0 = sg.tile([128, T], fp32)
            xs1 = sg.tile([128, T], fp32)
            y0 = sg.tile([128, J], fp32)
            y1 = sg.tile([128, J], fp32)
            wt0 = sg.tile([128, C], f32r)
            wt1 = sg.tile([128, C], f32r)
            gam = sg.tile([128, 2], fp32)
            bet = sg.tile([128, 2], fp32)
            cbv = sg.tile([128, 2], fp32)
            ident = sg.tile([128, 128], fp32)
            stats = sg.tile([128, 4, NT], fp32)
            astat = sg.tile([128, 4], fp32)
            scales = sg.tile([128, 2], fp32)
            shifts = sg.tile([128, 2], fp32)
            scr = sg.tile([128, JT], fp32)

            cc_in = dp.tile([128, 4], fp32)
            cc_out = dp.tile([128, 4], fp32, addr_space="Shared")

            # correlation operands first: the first j-tile only needs r
            nc.sync.dma_start(out=r0, in_=r_d[0:128, :])
            nc.sync.dma_start(out=r1, in_=r_d[128:256, :])
            nc.sync.dma_start(out=xs0, in_=xs_d[0:128, :])
            nc.sync.dma_start(out=xs1, in_=xs_d[128:256, :])
            nc.sync.dma_start(out=wt0, in_=wt_d[0:128, :])
            nc.sync.dma_start(out=wt1, in_=wt_d[128:256, :])
            nc.sync.dma_start(out=gam[:, 0:1], in_=gam_d[0:128])
            nc.sync.dma_start(out=gam[:, 1:2], in_=gam_d[128:256])
            nc.sync.dma_start(out=bet[:, 0:1], in_=bet_d[0:128])
            nc.sync.dma_start(out=bet[:, 1:2], in_=bet_d[128:256])
            nc.sync.dma_start(out=cbv[:, 0:1], in_=cb_d[0:128])
            nc.sync.dma_start(out=cbv[:, 1:2], in_=cb_d[128:256])

            make_identity(nc, ident)

            xs_t = (xs0, xs1)

            with tc.tile_pool(name="spool", bufs=2) as sp, \
                 tc.tile_pool(name="work", bufs=2) as wk, \
                 tc.tile_pool(name="gatp", bufs=2) as gp, \
                 tc.tile_pool(name="pp", bufs=6, space="PSUM") as pp, \
                 tc.tile_pool(name="pt", bufs=2, space="PSUM") as pt:

                for jt in range(NT):
                    f = jt // 7                      # local frame of this j-tile
                    j0 = jt * JT
                    s = sp.tile([JT, TS], fp32, tag="s")

                    # ---- correlation matmuls, frame-rotated column order
                    # chunks of 392 (= half frame): one PSUM bank each
                    for ci in range(14):
                        g = (f + 1 + ci // 2) % SNIP  # source frame for chunk
                        gc = g * F + (ci % 2) * CW
                        ps = pp.tile([JT, CW], fp32, tag="ps")
                        nc.tensor.matmul(ps, r0[:, j0:j0 + JT],
                                         r0[:, gc:gc + CW], start=True, stop=False)
                        nc.tensor.matmul(ps, r1[:, j0:j0 + JT],
                                         r1[:, gc:gc + CW], start=False, stop=True)
                        # drain PSUM -> s (ACT engine)
                        nc.scalar.copy(s[:, ci * CW:(ci + 1) * CW], ps[:])

                    if dbg and jt == 0:
                        nc.sync.dma_start(out=dbg_s[:], in_=s)

                    # ---- top-8 values + indices over the full 5488 (exact fp32)
                    t8 = wk.tile([JT, 8], fp32, tag="t8")
                    i8 = wk.tile([JT, 8], u16, tag="i8")
                    nc.vector.max(out=t8, in_=s)
                    nc.vector.max_index(out=i8, in_max=t8, in_values=s)

                    # searched col c -> clip col t = ((f+1)*784 + c) mod 6272
                    fall = wk.tile([JT, 8], fp32, tag="fall")
                    msk = wk.tile([JT, 8], fp32, tag="msk")
                    fin = wk.tile([JT, 8], fp32, tag="fin")
                    fdup = wk.tile([JT, 8], fp32, tag="fdup")
                    nc.vector.tensor_copy(fall, i8)            # u16 -> fp32
                    nc.vector.tensor_scalar_add(fall, fall, float((f + 1) * F))
                    nc.vector.tensor_scalar(out=msk, in0=fall, scalar1=float(T),
                                            scalar2=None, op0=Alu.is_ge)
                    nc.vector.scalar_tensor_tensor(out=fin, in0=msk,
                                                   scalar=float(-T), in1=fall,
                                                   op0=Alu.mult, op1=Alu.add)
                    nc.vector.tensor_copy(fdup[:, 0:5], fin[:, 0:5])
                    nc.vector.tensor_copy(fdup[:, 5:8],
                                          fin[:, 0:1].to_broadcast([JT, 3]))

                    if dbg and jt == 0:
                        nc.sync.dma_start(out=dbg_t8[:], in_=t8)
                        nc.sync.dma_start(out=dbg_i8[:], in_=i8)
                        nc.sync.dma_start(out=dbg_fin[:], in_=fin)

                    # ---- wrapped int16 index list for ap_gather
                    trp = pt.tile([8, JT], fp32, tag="tr")
                    nc.tensor.transpose(trp, fdup, ident[0:JT, 0:JT])
                    trs = wk.tile([8, JT], i16, tag="trs")
                    nc.vector.tensor_copy(trs, trp)
                    w16 = wk.tile([128, GAT // 16], i16, tag="w16")
                    trv = trs.rearrange("p (m two) -> p m two", two=2)
                    nc.sync.dma_start(out=w16[0:8, :], in_=trv[:, :, 0])
                    nc.sync.dma_start(out=w16[8:16, :], in_=trv[:, :, 1])
                    nc.sync.dma_start(out=w16[16:32, :], in_=w16[0:16, :])
                    nc.sync.dma_start(out=w16[32:64, :], in_=w16[0:32, :])
                    nc.sync.dma_start(out=w16[64:128, :], in_=w16[0:64, :])
                    if dbg and jt == 0:
                        nc.sync.dma_start(out=dbg_w16[:], in_=w16)

                    # ---- gather + max over the 5 picks (+3 dups), y stays in SBUF
                    y_t = (y0, y1)
                    for c in range(2):
                        gat = gp.tile([128, GAT], fp32, tag=f"gat{c}")
                        nc.gpsimd.ap_gather(out_ap=gat[:], in_ap=xs_t[c][:],
                                            idxs_ap=w16[:], channels=128,
                                            num_elems=T, d=1, num_idxs=GAT)
                        yt = y_t[c][:, j0:j0 + JT]
                        gv = gat.rearrange("p (j k) -> p j k", k=8)
                        nc.vector.tensor_reduce(out=yt, in_=gv, axis=Ax.X,
                                                op=Alu.max)
                        # batchnorm partial sums (ACT accumulator)
                        nc.scalar.activation(scr, yt, Act.Identity,
                                             accum_out=stats[:, 2 * c, jt:jt + 1])
                        nc.scalar.activation(scr, yt, Act.Square,
                                             accum_out=stats[:, 2 * c + 1, jt:jt + 1])
                        if dbg and jt == 0 and c == 0:
                            nc.sync.dma_start(out=dbg_yt[:], in_=yt)

            # ---- global batchnorm stats (allreduce over the 8 cores)
            nc.vector.tensor_reduce(out=astat, in_=stats, axis=Ax.X, op=Alu.add)
            nc.sync.dma_start(out=cc_in[:], in_=astat)
            nc.gpsimd.collective_compute(
                "AllReduce", Alu.add,
                replica_groups=[list(range(num_cores))],
                ins=[cc_in[:].opt()], outs=[cc_out[:].opt()])
            nc.sync.dma_start(out=astat, in_=cc_out[:])

            with tc.tile_pool(name="bnw", bufs=1) as bw:
                mean = bw.tile([128, 2], fp32)
                ex2 = bw.tile([128, 2], fp32)
                var = bw.tile([128, 2], fp32)
                std = bw.tile([128, 2], fp32)
                rstd = bw.tile([128, 2], fp32)
                vv = astat.rearrange("p (c m) -> p c m", m=2)
                nc.vector.tensor_scalar_mul(mean, vv[:, :, 0], 1.0 / NTOT)
                nc.vector.tensor_scalar_mul(ex2, vv[:, :, 1], 1.0 / NTOT)
                nc.vector.tensor_tensor(out=var, in0=mean, in1=mean, op=Alu.mult)
                nc.vector.tensor_sub(var, ex2, var)
                nc.vector.tensor_scalar_add(var, var, 1e-5)
                nc.scalar.sqrt(std, var)
                nc.vector.reciprocal(rstd, std)
                nc.vector.tensor_tensor(out=scales, in0=gam, in1=rstd, op=Alu.mult)
                nc.vector.tensor_tensor(out=shifts, in0=mean, in1=scales,
                                        op=Alu.mult)
                nc.vector.tensor_sub(shifts, bet, shifts)

            # ---- BN apply + relu + 1x1 conv + identity + store (y from SBUF)
            with tc.tile_pool(name="zp", bufs=2) as zp, \
                 tc.tile_pool(name="cp", bufs=2, space="PSUM") as cp:
                for ci in range(8):
                    c0 = ci * CW
                    z0 = zp.tile([128, CW], f32r, tag="z0")
                    z1 = zp.tile([128, CW], f32r, tag="z1")
                    nc.scalar.activation(z0, y0[:, c0:c0 + CW], Act.Relu,
                                         bias=shifts[:, 0:1], scale=scales[:, 0:1])
                    nc.scalar.activation(z1, y1[:, c0:c0 + CW], Act.Relu,
                                         bias=shifts[:, 1:2], scale=scales[:, 1:2])
                    fr, fc = divmod(ci, 2)
                    for ot in range(2):
                        o0 = ot * 128
                        cps = cp.tile([128, CW], fp32, tag="cps")
                        nc.tensor.matmul(cps, wt0[:, o0:o0 + 128], z0[:],
                                         start=True, stop=False)
                        nc.tensor.matmul(cps, wt1[:, o0:o0 + 128], z1[:],
                                         start=False, stop=True)
                        osb = zp.tile([128, CW], fp32, tag=f"osb{ot}")
                        nc.vector.scalar_tensor_tensor(
                            out=osb, in0=cps, scalar=cbv[:, ot:ot + 1],
                            in1=xs_t[ot][:, c0:c0 + CW], op0=Alu.add, op1=Alu.add)
                        nc.sync.dma_start(
                            out=out_d[fr, o0:o0 + 128, fc * CW:(fc + 1) * CW],
                            in_=osb)

    nc.finalize()
    return nc


def _get_nc(num_cores):
    if num_cores not in _CACHE:
        _CACHE[num_cores] = _build(num_cores)
    return _CACHE[num_cores]


def _prep_core_inputs(x, conv_w, gamma, beta, conv_b):
    """Build the 8 per-core input dicts from the full problem inputs."""
    xs_all = np.ascontiguousarray(
        x.reshape(4, SNIP, C, F).transpose(0, 2, 1, 3).reshape(4, C, T))
    wt = _round_f32r(np.ascontiguousarray(conv_w.T))
    maps = []
    for k in range(NCORES):
        b, h = divmod(k, 2)
        xs = xs_all[b]
        if h:
            xs = np.ascontiguousarray(
                np.concatenate((xs[:, J:], xs[:, :J]), axis=1))
        nrm = np.sqrt((xs * xs).sum(0, dtype=np.float32))
        xn = xs * (1.0 / nrm)[None, :].astype(np.float32)
        r = _round_f32r(xn)
        maps.append({
            "xs": xs,
            "xr": r,
            "wt": wt,
            "gam": np.ascontiguousarray(gamma, np.float32),
            "bet": np.ascontiguousarray(beta, np.float32),
            "cb": np.ascontiguousarray(conv_b, np.float32),
        })
    return maps


def kernel(x, gamma, beta, conv_w, conv_b, snip):
    assert int(snip) == SNIP and x.shape == (32, C, 28, 28)
    from concourse.bass_utils import run_bass_kernel_spmd

    x = np.ascontiguousarray(x, np.float32)
    maps = _prep_core_inputs(x, np.asarray(conv_w, np.float32),
                             np.asarray(gamma, np.float32),
                             np.asarray(beta, np.float32),
                             np.asarray(conv_b, np.float32))
    nc = _get_nc(NCORES)
    res = run_bass_kernel_spmd(nc, maps, list(range(NCORES))).results
    out = np.empty((32, C, F), np.float32)
    for k in range(NCORES):
        out[4 * k:4 * k + 4] = res[k]["out"]
    return out.reshape(32, C, 28, 28)


# revision 6
# speedup vs baseline: 2.7872x; 2.5359x over previous
"""Trainium2 Bass kernel for nn_Correspondence (retrieval_knn).

Pipeline per clip (B=4 clips, snip=8 frames of 28x28, C=256):
  xs = [C, THW=6272] per clip; corr = cosine similarity over channels;
  per column j: top-5 rows t (same-frame block excluded) -> gather xs cols,
  max over the 5 -> y; global BatchNorm (training stats) + relu -> 1x1 conv
  -> + identity.

Sharding: 8 cores = 4 clips x 2 column-halves. Each core gets its clip's
xs ROTATED by half the frames so its own j-range is local columns [0,3136)
— the same SPMD program runs on all cores. Same-frame masking is handled
by *never computing* the own-frame columns (frame-rotated chunk order).

Precision: the correlation matmul runs in float32r only (12-bit mantissa,
fp32 accumulate). On the actual seed-0 data this flips the top-5 set on
~43/25088 columns vs exact fp32, giving end-to-end rel err ~5e-3 — well
under the 2e-2 gate. Gather/BN run in exact fp32; the 1x1 conv uses f32r.

Gather strategy: ap_gather costs ~27ns/index (4-idx read requests on the
Q7), which made it the whole-kernel bottleneck. Instead the top-5 columns
are pulled with gpsimd.dma_gather from a host-prepared TRANSPOSED copy
xsT [T, C] in DRAM: each int16 index fetches one 1 KiB row (a full
256-channel column) via SWDGE descriptors at DMA line rate. Index order
i = 16*e + p with e = psi + 8*s, so gather-output row 16*psi+p == j-j0
(identity within a 128-j block) and the 5 candidates of each j land in one
partition at stride 256 — a single strided DVE max-reduce yields yT
[j, c], which two PE transposes turn back into y [c, j] in SBUF.
The wrapped index strip is built from per-tile PE transposes of the fin
indices with 5 strided DMAs per wave. Blocks are wave-pipelined against
the top-k loop. BN stats accumulate per-block on ACT; one AllReduce at
the end, then BN+relu+1x1 conv straight out of SBUF.
"""
import sys, os
import numpy as np

for _p in ("/opt/trn_rl_repo", "/root/.axon_site/_ro/trn_rl_repo"):
    if os.path.isdir(_p) and _p not in sys.path:
        sys.path.insert(0, _p)
        break

# ---------------- problem constants (hardcoded) ----------------
C = 256          # channels
SNIP = 8         # frames per clip
F = 784          # 28*28
T = SNIP * F     # 6272 columns per clip
J = T // 2       # 3136 columns handled per core
JT = 112         # j-tile rows (one PE M-tile; 112*7 = 784 -> tiles never span frames)
NT = J // JT     # 28 j-tiles
TS = 7 * F       # 5488 searched columns per j-tile (own frame excluded)
KTOP = 5
NCORES = 8
NTOT = 32 * F    # batchnorm count = BS*H*W = 25088
CW = 392         # matmul chunk width (one PSUM bank)
NB = 25          # gather blocks of 128 j (last block overlaps, j0=3008)
BW = KTOP * 8    # 40 wrapped-idx elements per block per partition-row

_CACHE = {}


def _round_f32r(x):
    """Round-to-nearest-even to f32r (low 12 mantissa bits zeroed)."""
    b = np.ascontiguousarray(x, np.float32).view(np.uint32)
    low = b & np.uint32(0xFFF)
    add = (low > 0x800) | ((low == 0x800) & (((b >> 12) & 1) == 1))
    b = (b & ~np.uint32(0xFFF)) + (add.astype(np.uint32) << 12)
    return b.view(np.float32)


def _build(num_cores, dbg=False):
    import concourse.bass as bass
    import concourse.mybir as mybir
    import concourse.tile as tile
    from concourse import bacc
    from concourse.masks import make_identity

    fp32 = mybir.dt.float32
    f32r = mybir.dt.float32r
    i16 = mybir.dt.int16
    u16 = mybir.dt.uint16
    Alu = mybir.AluOpType
    Act = mybir.ActivationFunctionType
    Ax = mybir.AxisListType

    nc = bacc.Bacc("TRN2", target_bir_lowering=False, debug=False,
                   num_devices=num_cores)

    xs_d = nc.declare_dram_parameter("xs", [C, T], fp32, isOutput=False)
    xst_d = nc.declare_dram_parameter("xst", [T, C], fp32, isOutput=False)
    r_d = nc.declare_dram_parameter("xr", [C, T], f32r, isOutput=False)
    wt_d = nc.declare_dram_parameter("wt", [C, C], f32r, isOutput=False)
    gam_d = nc.declare_dram_parameter("gam", [C], fp32, isOutput=False)
    bet_d = nc.declare_dram_parameter("bet", [C], fp32, isOutput=False)
    cb_d = nc.declare_dram_parameter("cb", [C], fp32, isOutput=False)
    out_d = nc.declare_dram_parameter("out", [4, C, F], fp32, isOutput=True)
    if dbg:
        dbg_fin = nc.declare_dram_parameter("dbg_fin", [JT, 8], fp32, isOutput=True)
        dbg_w16 = nc.declare_dram_parameter("dbg_w16", [128, NB * BW], i16, isOutput=True)
        dbg_gg = nc.declare_dram_parameter("dbg_gg", [128, KTOP, C], fp32, isOutput=True)
        dbg_yt = nc.declare_dram_parameter("dbg_yt", [128, C], fp32, isOutput=True)

    # gather block j-origins: 24 full blocks + overlap block at 3008
    bj0 = [min(128 * b, J - 128) for b in range(NB)]
    # emission plan: after tile `at`, wrapped-idx strip for blocks [b0,b1) is built
    waves = {7: (0, 6), 13: (6, 12), 20: (12, 18), 27: (18, 25)}

    with tile.TileContext(nc) as tc:
        with tc.tile_pool(name="singles", bufs=1) as sg, \
             tc.tile_pool(name="dram", bufs=1, space="DRAM") as dp:
            # ---- persistent inputs in SBUF
            r0 = sg.tile([128, T], f32r)
            r1 = sg.tile([128, T], f32r)
            xs0 = sg.tile([128, T], fp32)
            xs1 = sg.tile([128, T], fp32)
            y0 = sg.tile([128, J], fp32)
            y1 = sg.tile([128, J], fp32)
            wt0 = sg.tile([128, C], f32r)
            wt1 = sg.tile([128, C], f32r)
            gam = sg.tile([128, 2], fp32)
            bet = sg.tile([128, 2], fp32)
            cbv = sg.tile([128, 2], fp32)
            ident = sg.tile([128, 128], fp32)
            stats = sg.tile([128, 4, NB], fp32)
            astat = sg.tile([128, 4], fp32)
            scales = sg.tile([128, 2], fp32)
            shifts = sg.tile([128, 2], fp32)
            scr = sg.tile([128, 128], fp32)
            trps = sg.tile([8, J], i16)          # transposed top-5 index strip
            w16a = sg.tile([128, NB * BW], i16)  # wrapped idx list, replicated

            cc_in = dp.tile([128, 4], fp32)
            cc_out = dp.tile([128, 4], fp32, addr_space="Shared")

            # correlation operands first: the first j-tile only needs r
            nc.sync.dma_start(out=r0, in_=r_d[0:128, :])
            nc.sync.dma_start(out=r1, in_=r_d[128:256, :])
            nc.sync.dma_start(out=xs0, in_=xs_d[0:128, :])
            nc.sync.dma_start(out=xs1, in_=xs_d[128:256, :])
            nc.sync.dma_start(out=wt0, in_=wt_d[0:128, :])
            nc.sync.dma_start(out=wt1, in_=wt_d[128:256, :])
            nc.sync.dma_start(out=gam[:, 0:1], in_=gam_d[0:128])
            nc.sync.dma_start(out=gam[:, 1:2], in_=gam_d[128:256])
            nc.sync.dma_start(out=bet[:, 0:1], in_=bet_d[0:128])
            nc.sync.dma_start(out=bet[:, 1:2], in_=bet_d[128:256])
            nc.sync.dma_start(out=cbv[:, 0:1], in_=cb_d[0:128])
            nc.sync.dma_start(out=cbv[:, 1:2], in_=cb_d[128:256])

            make_identity(nc, ident)

            with tc.tile_pool(name="spool", bufs=2) as sp, \
                 tc.tile_pool(name="work", bufs=2) as wk, \
                 tc.tile_pool(name="gatp", bufs=2) as gp, \
                 tc.tile_pool(name="pp", bufs=4, space="PSUM") as pp, \
                 tc.tile_pool(name="pt", bufs=2, space="PSUM") as pt, \
                 tc.tile_pool(name="py", bufs=2, space="PSUM") as py:

                def emit_w16(b0, b1):
                    """Build wrapped idx list for blocks [b0,b1): w16a[p, b*40+8s+psi]
                    = trps[s, bj0[b] + 8*p + psi], then replicate 16->128.
                    (Gather-out row 16*psi+p then holds j = j0+8p+psi; the
                    block drain un-permutes via strided APs.)"""
                    for b in range(b0, b1):
                        for s in range(KTOP):
                            e0 = b * BW + 8 * s
                            nc.sync.dma_start(
                                out=w16a[0:16, e0:e0 + 8],
                                in_=trps[s:s + 1, bj0[b]:bj0[b] + 128])
                    lo, hi = b0 * BW, b1 * BW
                    nc.sync.dma_start(out=w16a[16:32, lo:hi], in_=w16a[0:16, lo:hi])
                    nc.sync.dma_start(out=w16a[32:64, lo:hi], in_=w16a[0:32, lo:hi])
                    nc.sync.dma_start(out=w16a[64:128, lo:hi], in_=w16a[0:64, lo:hi])

                def emit_block(b):
                    """Gather 5 full columns per j for one 128-j block, reduce,
                    transpose back to [c, j], accumulate BN stats."""
                    j0b = bj0[b]
                    gg = gp.tile([128, KTOP, C], fp32, tag="gg")
                    nc.gpsimd.dma_gather(gg[:], xst_d[:, :],
                                         w16a[:, b * BW:(b + 1) * BW],
                                         KTOP * 128, KTOP * 128, C)
                    yt = gp.tile([128, C], fp32, tag="yt")
                    gv = gg.rearrange("p s c -> p c s")
                    nc.vector.tensor_reduce(out=yt, in_=gv, axis=Ax.X, op=Alu.max)
                    ytr = py.tile([128, 2, 128], fp32, tag="ytr")
                    nc.tensor.transpose(ytr[:, 0, :], yt[:, 0:128], ident)
                    nc.tensor.transpose(ytr[:, 1, :], yt[:, 128:256], ident)
                    # gather row r holds j = j0 + 8*(r%16) + r//16 -> un-permute
                    for c, yy in ((0, y0), (1, y1)):
                        dst = yy[:, j0b:j0b + 128].rearrange(
                            "c (p psi) -> c p psi", p=16)
                        src = ytr[:, c, :].rearrange(
                            "c (psi p) -> c p psi", psi=8)
                        nc.scalar.copy(dst, src)
                    # BN partial sums; overlap block only contributes its new cols
                    st0 = 64 if b == NB - 1 else 0
                    for c, yy in ((0, y0), (1, y1)):
                        sl = yy[:, j0b + st0:j0b + 128]
                        nc.scalar.activation(scr[:, st0:128], sl, Act.Identity,
                                             accum_out=stats[:, 2 * c, b:b + 1])
                        nc.scalar.activation(scr[:, st0:128], sl, Act.Square,
                                             accum_out=stats[:, 2 * c + 1, b:b + 1])
                    if dbg and b == 0:
                        nc.sync.dma_start(out=dbg_gg[:], in_=gg)
                        nc.sync.dma_start(out=dbg_yt[:], in_=yt)

                pending = []
                for jt in range(NT):
                    f = jt // 7                      # local frame of this j-tile
                    j0 = jt * JT
                    s = sp.tile([JT, TS], fp32, tag="s")

                    # ---- correlation matmuls, frame-rotated column order
                    for ci in range(14):
                        g = (f + 1 + ci // 2) % SNIP  # source frame for chunk
                        gc = g * F + (ci % 2) * CW
                        ps = pp.tile([JT, CW], fp32, tag="ps")
                        nc.tensor.matmul(ps, r0[:, j0:j0 + JT],
                                         r0[:, gc:gc + CW], start=True, stop=False)
                        nc.tensor.matmul(ps, r1[:, j0:j0 + JT],
                                         r1[:, gc:gc + CW], start=False, stop=True)
                        nc.scalar.copy(s[:, ci * CW:(ci + 1) * CW], ps[:])

                    # ---- top-8 values + indices over the full 5488 (exact fp32)
                    t8 = wk.tile([JT, 8], fp32, tag="t8")
                    i8 = wk.tile([JT, 8], u16, tag="i8")
                    nc.vector.max(out=t8, in_=s)
                    nc.vector.max_index(out=i8, in_max=t8, in_values=s)

                    # searched col c -> clip col t = ((f+1)*784 + c) mod 6272
                    fall = wk.tile([JT, 8], fp32, tag="fall")
                    msk = wk.tile([JT, 8], fp32, tag="msk")
                    fin = wk.tile([JT, 8], fp32, tag="fin")
                    nc.vector.tensor_copy(fall, i8)            # u16 -> fp32
                    nc.vector.tensor_scalar_add(fall, fall, float((f + 1) * F))
                    nc.vector.tensor_scalar(out=msk, in0=fall, scalar1=float(T),
                                            scalar2=None, op0=Alu.is_ge)
                    nc.vector.scalar_tensor_tensor(out=fin, in0=msk,
                                                   scalar=float(-T), in1=fall,
                                                   op0=Alu.mult, op1=Alu.add)
                    if dbg and jt == 0:
                        nc.sync.dma_start(out=dbg_fin[:], in_=fin)

                    # ---- transpose to the index strip (slots on partitions)
                    trp = pt.tile([8, JT], fp32, tag="tr")
                    nc.tensor.transpose(trp, fin, ident[0:JT, 0:JT])
                    nc.vector.tensor_copy(trps[:, j0:j0 + JT], trp)  # fp32 -> i16

                    # ---- wave-pipelined gather blocks
                    if jt in waves:
                        emit_w16(*waves[jt])
                        pending.extend(range(*waves[jt]))
                        if dbg and jt == 27:
                            nc.sync.dma_start(out=dbg_w16[:], in_=w16a)
                    if pending and jt >= 8:
                        emit_block(pending.pop(0))
                while pending:
                    emit_block(pending.pop(0))

            # ---- global batchnorm stats (allreduce over the 8 cores)
            nc.vector.tensor_reduce(out=astat, in_=stats, axis=Ax.X, op=Alu.add)
            nc.sync.dma_start(out=cc_in[:], in_=astat)
            nc.gpsimd.collective_compute(
                "AllReduce", Alu.add,
                replica_groups=[list(range(num_cores))],
                ins=[cc_in[:].opt()], outs=[cc_out[:].opt()])
            nc.sync.dma_start(out=astat, in_=cc_out[:])

            with tc.tile_pool(name="bnw", bufs=1) as bw:
                mean = bw.tile([128, 2], fp32)
                ex2 = bw.tile([128, 2], fp32)
                var = bw.tile([128, 2], fp32)
                std = bw.tile([128, 2], fp32)
                rstd = bw.tile([128, 2], fp32)
                vv = astat.rearrange("p (c m) -> p c m", m=2)
                nc.vector.tensor_scalar_mul(mean, vv[:, :, 0], 1.0 / NTOT)
                nc.vector.tensor_scalar_mul(ex2, vv[:, :, 1], 1.0 / NTOT)
                nc.vector.tensor_tensor(out=var, in0=mean, in1=mean, op=Alu.mult)
                nc.vector.tensor_sub(var, ex2, var)
                nc.vector.tensor_scalar_add(var, var, 1e-5)
                nc.scalar.sqrt(std, var)
                nc.vector.reciprocal(rstd, std)
                nc.vector.tensor_tensor(out=scales, in0=gam, in1=rstd, op=Alu.mult)
                nc.vector.tensor_tensor(out=shifts, in0=mean, in1=scales,
                                        op=Alu.mult)
                nc.vector.tensor_sub(shifts, bet, shifts)

            # ---- BN apply + relu + 1x1 conv + identity + store (y from SBUF)
            xs_t = (xs0, xs1)
            with tc.tile_pool(name="zp", bufs=2) as zp, \
                 tc.tile_pool(name="cp", bufs=2, space="PSUM") as cp:
                for ci in range(8):
                    c0 = ci * CW
                    z0 = zp.tile([128, CW], f32r, tag="z0")
                    z1 = zp.tile([128, CW], f32r, tag="z1")
                    nc.scalar.activation(z0, y0[:, c0:c0 + CW], Act.Relu,
                                         bias=shifts[:, 0:1], scale=scales[:, 0:1])
                    nc.scalar.activation(z1, y1[:, c0:c0 + CW], Act.Relu,
                                         bias=shifts[:, 1:2], scale=scales[:, 1:2])
                    fr, fc = divmod(ci, 2)
                    for ot in range(2):
                        o0 = ot * 128
                        cps = cp.tile([128, CW], fp32, tag="cps")
                        nc.tensor.matmul(cps, wt0[:, o0:o0 + 128], z0[:],
                                         start=True, stop=False)
                        nc.tensor.matmul(cps, wt1[:, o0:o0 + 128], z1[:],
                                         start=False, stop=True)
                        osb = zp.tile([128, CW], fp32, tag=f"osb{ot}")
                        nc.vector.scalar_tensor_tensor(
                            out=osb, in0=cps, scalar=cbv[:, ot:ot + 1],
                            in1=xs_t[ot][:, c0:c0 + CW], op0=Alu.add, op1=Alu.add)
                        nc.sync.dma_start(
                            out=out_d[fr, o0:o0 + 128, fc * CW:(fc + 1) * CW],
                            in_=osb)

    nc.finalize()
    return nc


def _get_nc(num_cores):
    if num_cores not in _CACHE:
        _CACHE[num_cores] = _build(num_cores)
    return _CACHE[num_cores]


def _prep_core_inputs(x, conv_w, gamma, beta, conv_b):
    """Build the 8 per-core input dicts from the full problem inputs."""
    xs_all = np.ascontiguousarray(
        x.reshape(4, SNIP, C, F).transpose(0, 2, 1, 3).reshape(4, C, T))
    wt = _round_f32r(np.ascontiguousarray(conv_w.T))
    maps = []
    for k in range(NCORES):
        b, h = divmod(k, 2)
        xs = xs_all[b]
        if h:
            xs = np.ascontiguousarray(
                np.concatenate((xs[:, J:], xs[:, :J]), axis=1))
        nrm = np.sqrt((xs * xs).sum(0, dtype=np.float32))
        xn = xs * (1.0 / nrm)[None, :].astype(np.float32)
        r = _round_f32r(xn)
        maps.append({
            "xs": xs,
            "xst": np.ascontiguousarray(xs.T),
            "xr": r,
            "wt": wt,
            "gam": np.ascontiguousarray(gamma, np.float32),
            "bet": np.ascontiguousarray(beta, np.float32),
            "cb": np.ascontiguousarray(conv_b, np.float32),
        })
    return maps


def kernel(x, gamma, beta, conv_w, conv_b, snip):
    assert int(snip) == SNIP and x.shape == (32, C, 28, 28)
    from concourse.bass_utils import run_bass_kernel_spmd

    x = np.ascontiguousarray(x, np.float32)
    maps = _prep_core_inputs(x, np.asarray(conv_w, np.float32),
                             np.asarray(gamma, np.float32),
                             np.asarray(beta, np.float32),
                             np.asarray(conv_b, np.float32))
    nc = _get_nc(NCORES)
    res = run_bass_kernel_spmd(nc, maps, list(range(NCORES))).results
    out = np.empty((32, C, F), np.float32)
    for k in range(NCORES):
        out[4 * k:4 * k + 4] = res[k]["out"]
    return out.reshape(32, C, 28, 28)


# revision 16
# speedup vs baseline: 3.0942x; 1.1101x over previous
"""Trainium2 Bass kernel for nn_Correspondence (retrieval_knn).

Pipeline per clip (B=4 clips, snip=8 frames of 28x28, C=256):
  xs = [C, THW=6272] per clip; corr = cosine similarity over channels;
  per column j: top-5 rows t (same-frame block excluded) -> gather xs cols,
  max over the 5 -> y; global BatchNorm (training stats) + relu -> 1x1 conv
  -> + identity.

Sharding: 8 cores = 4 clips x 2 column-halves. Each core gets its clip's
xs ROTATED by half the frames so its own j-range is local columns [0,3136)
— the same SPMD program runs on all cores. Same-frame masking is handled
by *never computing* the own-frame columns (frame-rotated chunk order).

Precision: the correlation matmul runs in float32r only (12-bit mantissa,
fp32 accumulate). On the actual seed-0 data this flips the top-5 set on
~43/25088 columns vs exact fp32, giving end-to-end rel err ~5e-3 — well
under the 2e-2 gate. Gather/BN run in exact fp32; the 1x1 conv uses f32r.

Gather strategy: ap_gather costs ~27ns/index (4-idx read requests on the
Q7), which made it the whole-kernel bottleneck. Instead the top-5 columns
are pulled with gpsimd.dma_gather from a host-prepared TRANSPOSED copy
xsT [T, C] in DRAM: each int16 index fetches one 1 KiB row (a full
256-channel column) via SWDGE descriptors at DMA line rate. Index order
i = 16*e + p with e = psi + 8*s, so gather-output row 16*psi+p == j-j0
(identity within a 128-j block) and the 5 candidates of each j land in one
partition at stride 256 — a single strided DVE max-reduce yields yT
[j, c], which two PE transposes turn back into y [c, j] in SBUF.
The wrapped index strip is built from per-tile PE transposes of the fin
indices with 5 strided DMAs per wave. Blocks are wave-pipelined against
the top-k loop. BN stats accumulate per-block on ACT; one AllReduce at
the end, then BN+relu+1x1 conv straight out of SBUF.
"""
import sys, os
import numpy as np

for _p in ("/opt/trn_rl_repo", "/root/.axon_site/_ro/trn_rl_repo"):
    if os.path.isdir(_p) and _p not in sys.path:
        sys.path.insert(0, _p)
        break

# ---------------- problem constants (hardcoded) ----------------
C = 256          # channels
SNIP = 8         # frames per clip
F = 784          # 28*28
T = SNIP * F     # 6272 columns per clip
J = T // 2       # 3136 columns handled per core
JT = 112         # j-tile rows (one PE M-tile; 112*7 = 784 -> tiles never span frames)
NT = J // JT     # 28 j-tiles
TS = 7 * F       # 5488 searched columns per j-tile (own frame excluded)
KTOP = 5
NCORES = 8
NTOT = 32 * F    # batchnorm count = BS*H*W = 25088
CW = 392         # matmul chunk width (one PSUM bank)
NB = 25          # gather blocks of 128 j (last block overlaps, j0=3008)
BW = KTOP * 8    # 40 wrapped-idx elements per block per partition-row

_CACHE = {}


def _round_f32r(x):
    """Round-to-nearest-even to f32r (low 12 mantissa bits zeroed)."""
    b = np.ascontiguousarray(x, np.float32).view(np.uint32)
    low = b & np.uint32(0xFFF)
    add = (low > 0x800) | ((low == 0x800) & (((b >> 12) & 1) == 1))
    b = (b & ~np.uint32(0xFFF)) + (add.astype(np.uint32) << 12)
    return b.view(np.float32)


def _build(num_cores, dbg=False):
    import concourse.bass as bass
    import concourse.mybir as mybir
    import concourse.tile as tile
    from concourse import bacc
    from concourse.masks import make_identity

    fp32 = mybir.dt.float32
    f32r = mybir.dt.float32r
    i16 = mybir.dt.int16
    u16 = mybir.dt.uint16
    Alu = mybir.AluOpType
    Act = mybir.ActivationFunctionType
    Ax = mybir.AxisListType

    nc = bacc.Bacc("TRN2", target_bir_lowering=False, debug=False,
                   num_devices=num_cores)

    xs_d = nc.declare_dram_parameter("xs", [C, T], fp32, isOutput=False)
    # doubled transposed copy: gather indices skip the mod-T wrap entirely
    xst_d = nc.declare_dram_parameter("xst", [2 * T, C], fp32, isOutput=False)
    r_d = nc.declare_dram_parameter("xr", [C, T], f32r, isOutput=False)
    wt_d = nc.declare_dram_parameter("wt", [C, C], f32r, isOutput=False)
    gam_d = nc.declare_dram_parameter("gam", [C], fp32, isOutput=False)
    bet_d = nc.declare_dram_parameter("bet", [C], fp32, isOutput=False)
    cb_d = nc.declare_dram_parameter("cb", [C], fp32, isOutput=False)
    out_d = nc.declare_dram_parameter("out", [4, C, F], fp32, isOutput=True)
    if dbg:
        dbg_fin = nc.declare_dram_parameter("dbg_fin", [JT, 8], fp32, isOutput=True)
        dbg_w16 = nc.declare_dram_parameter("dbg_w16", [128, NB * BW], i16, isOutput=True)
        dbg_gg = nc.declare_dram_parameter("dbg_gg", [128, KTOP, C], fp32, isOutput=True)
        dbg_yt = nc.declare_dram_parameter("dbg_yt", [128, C], fp32, isOutput=True)

    # gather block j-origins: 24 full blocks + overlap block at 3008
    bj0 = [min(128 * b, J - 128) for b in range(NB)]
    # emission plan: after tile `at`, wrapped-idx strip for blocks [b0,b1) is built
    waves = {3: (0, 2), 7: (2, 6), 13: (6, 12), 20: (12, 18),
             25: (18, 22), 27: (22, 25)}

    with tile.TileContext(nc) as tc:
        with tc.tile_pool(name="singles", bufs=1) as sg, \
             tc.tile_pool(name="dram", bufs=1, space="DRAM") as dp:
            # ---- persistent inputs in SBUF
            r0 = sg.tile([128, T], f32r)
            r1 = sg.tile([128, T], f32r)
            xs0 = sg.tile([128, T], fp32)
            xs1 = sg.tile([128, T], fp32)
            y0 = sg.tile([128, J], fp32)
            y1 = sg.tile([128, J], fp32)
            wt0 = sg.tile([128, C], f32r)
            wt1 = sg.tile([128, C], f32r)
            gam = sg.tile([128, 2], fp32)
            bet = sg.tile([128, 2], fp32)
            cbv = sg.tile([128, 2], fp32)
            ident = sg.tile([128, 128], fp32)
            stats = sg.tile([128, 4, NB], fp32)
            astat = sg.tile([128, 4], fp32)
            scales = sg.tile([128, 2], fp32)
            shifts = sg.tile([128, 2], fp32)
            scr = sg.tile([128, 128], fp32)
            trps = sg.tile([8, J], i16)          # transposed top-5 index strip
            w16a = sg.tile([128, NB * BW], i16)  # wrapped idx list, replicated

            cc_in = dp.tile([128, 4], fp32)
            cc_out = dp.tile([128, 4], fp32, addr_space="Shared")

            # correlation operands first: the first j-tile only needs r.
            # xs/wt/BN params are epilogue-only and load later (at jt==1).
            nc.sync.dma_start(out=r0, in_=r_d[0:128, :])
            nc.sync.dma_start(out=r1, in_=r_d[128:256, :])

            make_identity(nc, ident)
            bases = sg.tile([JT, 4], fp32)
            for ff in range(4):
                nc.vector.memset(bases[:, ff:ff + 1], float((ff + 1) * F))

            with tc.tile_pool(name="spool", bufs=2) as sp, \
                 tc.tile_pool(name="work", bufs=2) as wk, \
                 tc.tile_pool(name="gatp", bufs=2) as gp, \
                 tc.tile_pool(name="pp", bufs=4, space="PSUM") as pp, \
                 tc.tile_pool(name="pt", bufs=2, space="PSUM") as pt, \
                 tc.tile_pool(name="py", bufs=2, space="PSUM") as py:

                def emit_w16(b0, b1):
                    """Build wrapped idx list for blocks [b0,b1): w16a[p, b*40+8s+psi]
                    = trps[s, bj0[b] + 8*p + psi], then replicate 16->128.
                    (Gather-out row 16*psi+p then holds j = j0+8p+psi; the
                    block drain un-permutes via strided APs.)"""
                    for b in range(b0, b1):
                        for s in range(KTOP):
                            e0 = b * BW + 8 * s
                            nc.sync.dma_start(
                                out=w16a[0:16, e0:e0 + 8],
                                in_=trps[s:s + 1, bj0[b]:bj0[b] + 128])
                    lo, hi = b0 * BW, b1 * BW
                    nc.sync.dma_start(out=w16a[16:32, lo:hi], in_=w16a[0:16, lo:hi])
                    nc.sync.dma_start(out=w16a[32:64, lo:hi], in_=w16a[0:32, lo:hi])
                    nc.sync.dma_start(out=w16a[64:128, lo:hi], in_=w16a[0:64, lo:hi])

                def emit_block(b):
                    """Gather 5 full columns per j for one 128-j block, reduce,
                    transpose back to [c, j], accumulate BN stats."""
                    j0b = bj0[b]
                    gg = gp.tile([128, KTOP, C], fp32, tag="gg")
                    nc.gpsimd.dma_gather(gg[:], xst_d[:, :],
                                         w16a[:, b * BW:(b + 1) * BW],
                                         KTOP * 128, KTOP * 128, C)
                    # pairwise max tree on contiguous [128,C] slices (faster on
                    # DVE than one strided 5-way reduce)
                    m2 = gp.tile([128, 2, C], fp32, tag="m2")
                    yt = gp.tile([128, C], fp32, tag="yt")
                    nc.vector.tensor_tensor(out=m2.rearrange("p a c -> p (a c)"),
                                            in0=gg.rearrange("p s c -> p (s c)")[:, 0:2 * C],
                                            in1=gg.rearrange("p s c -> p (s c)")[:, 2 * C:4 * C],
                                            op=Alu.max)
                    nc.vector.tensor_tensor(out=yt, in0=m2[:, 0, :],
                                            in1=m2[:, 1, :], op=Alu.max)
                    nc.vector.tensor_tensor(out=yt, in0=yt, in1=gg[:, 4, :],
                                            op=Alu.max)
                    ytr = py.tile([128, 2, 128], fp32, tag="ytr")
                    nc.tensor.transpose(ytr[:, 0, :], yt[:, 0:128], ident)
                    nc.tensor.transpose(ytr[:, 1, :], yt[:, 128:256], ident)
                    # gather row r holds j = j0 + 8*(r%16) + r//16 -> un-permute
                    for c, yy in ((0, y0), (1, y1)):
                        dst = yy[:, j0b:j0b + 128].rearrange(
                            "c (p psi) -> c p psi", p=16)
                        src = ytr[:, c, :].rearrange(
                            "c (psi p) -> c p psi", psi=8)
                        nc.scalar.copy(dst, src)
                    # BN partial sums; overlap block only contributes its new cols
                    st0 = 64 if b == NB - 1 else 0
                    for c, yy in ((0, y0), (1, y1)):
                        sl = yy[:, j0b + st0:j0b + 128]
                        nc.scalar.activation(scr[:, st0:128], sl, Act.Identity,
                                             accum_out=stats[:, 2 * c, b:b + 1])
                        nc.scalar.activation(scr[:, st0:128], sl, Act.Square,
                                             accum_out=stats[:, 2 * c + 1, b:b + 1])
                    if dbg and b == 0:
                        nc.sync.dma_start(out=dbg_gg[:], in_=gg)
                        nc.sync.dma_start(out=dbg_yt[:], in_=yt)

                pending = []
                for jt in range(NT):
                    f = jt // 7                      # local frame of this j-tile
                    j0 = jt * JT
                    if jt == 1:
                        # epilogue-only loads, deferred off the startup path
                        nc.sync.dma_start(out=xs0, in_=xs_d[0:128, :])
                        nc.sync.dma_start(out=xs1, in_=xs_d[128:256, :])
                        nc.sync.dma_start(out=wt0, in_=wt_d[0:128, :])
                        nc.sync.dma_start(out=wt1, in_=wt_d[128:256, :])
                        nc.sync.dma_start(out=gam[:, 0:1], in_=gam_d[0:128])
                        nc.sync.dma_start(out=gam[:, 1:2], in_=gam_d[128:256])
                        nc.sync.dma_start(out=bet[:, 0:1], in_=bet_d[0:128])
                        nc.sync.dma_start(out=bet[:, 1:2], in_=bet_d[128:256])
                        nc.sync.dma_start(out=cbv[:, 0:1], in_=cb_d[0:128])
                        nc.sync.dma_start(out=cbv[:, 1:2], in_=cb_d[128:256])
                    s = sp.tile([JT, TS], fp32, tag="s")

                    # ---- correlation matmuls, frame-rotated column order
                    for ci in range(14):
                        g = (f + 1 + ci // 2) % SNIP  # source frame for chunk
                        gc = g * F + (ci % 2) * CW
                        ps = pp.tile([JT, CW], fp32, tag="ps")
                        nc.tensor.matmul(ps, r0[:, j0:j0 + JT],
                                         r0[:, gc:gc + CW], start=True, stop=False)
                        nc.tensor.matmul(ps, r1[:, j0:j0 + JT],
                                         r1[:, gc:gc + CW], start=False, stop=True)
                        nc.scalar.copy(s[:, ci * CW:(ci + 1) * CW], ps[:])

                    # ---- top-8 values + indices over the full 5488 (exact fp32)
                    t8 = wk.tile([JT, 8], fp32, tag="t8")
                    i8 = wk.tile([JT, 8], u16, tag="i8")
                    nc.vector.max(out=t8, in_=s)
                    nc.vector.max_index(out=i8, in_max=t8, in_values=s)

                    # searched col c -> doubled-xsT row (f+1)*784 + c (no wrap
                    # needed: xst is xs.T twice). Done on ACT to keep DVE free.
                    fin = wk.tile([JT, 8], fp32, tag="fin")
                    nc.scalar.activation(fin, i8, Act.Identity,
                                         bias=bases[:, f:f + 1])
                    if dbg and jt == 0:
                        nc.sync.dma_start(out=dbg_fin[:], in_=fin)

                    # ---- transpose to the index strip (slots on partitions)
                    trp = pt.tile([8, JT], fp32, tag="tr")
                    nc.tensor.transpose(trp, fin, ident[0:JT, 0:JT])
                    nc.scalar.copy(trps[:, j0:j0 + JT], trp)  # fp32 -> i16

                    # ---- wave-pipelined gather blocks
                    if jt in waves:
                        emit_w16(*waves[jt])
                        pending.extend(range(*waves[jt]))
                        if dbg and jt == 27:
                            nc.sync.dma_start(out=dbg_w16[:], in_=w16a)
                    if pending and jt >= 4:
                        emit_block(pending.pop(0))
                while pending:
                    emit_block(pending.pop(0))

            # ---- global batchnorm stats (allreduce over the 8 cores)
            nc.vector.tensor_reduce(out=astat, in_=stats, axis=Ax.X, op=Alu.add)
            nc.sync.dma_start(out=cc_in[:], in_=astat)
            nc.gpsimd.collective_compute(
                "AllReduce", Alu.add,
                replica_groups=[list(range(num_cores))],
                ins=[cc_in[:].opt()], outs=[cc_out[:].opt()])
            nc.sync.dma_start(out=astat, in_=cc_out[:])

            with tc.tile_pool(name="bnw", bufs=1) as bw:
                mean = bw.tile([128, 2], fp32)
                ex2 = bw.tile([128, 2], fp32)
                var = bw.tile([128, 2], fp32)
                std = bw.tile([128, 2], fp32)
                rstd = bw.tile([128, 2], fp32)
                vv = astat.rearrange("p (c m) -> p c m", m=2)
                nc.vector.tensor_scalar_mul(mean, vv[:, :, 0], 1.0 / NTOT)
                nc.vector.tensor_scalar_mul(ex2, vv[:, :, 1], 1.0 / NTOT)
                nc.vector.tensor_tensor(out=var, in0=mean, in1=mean, op=Alu.mult)
                nc.vector.tensor_sub(var, ex2, var)
                nc.vector.tensor_scalar_add(var, var, 1e-5)
                nc.scalar.sqrt(std, var)
                nc.vector.reciprocal(rstd, std)
                nc.vector.tensor_tensor(out=scales, in0=gam, in1=rstd, op=Alu.mult)
                nc.vector.tensor_tensor(out=shifts, in0=mean, in1=scales,
                                        op=Alu.mult)
                nc.vector.tensor_sub(shifts, bet, shifts)

            # ---- BN apply + relu + 1x1 conv + identity + store (y from SBUF)
            xs_t = (xs0, xs1)
            with tc.tile_pool(name="zp", bufs=2) as zp, \
                 tc.tile_pool(name="cp", bufs=2, space="PSUM") as cp:
                for ci in range(8):
                    c0 = ci * CW
                    z0 = zp.tile([128, CW], f32r, tag="z0")
                    z1 = zp.tile([128, CW], f32r, tag="z1")
                    nc.scalar.activation(z0, y0[:, c0:c0 + CW], Act.Relu,
                                         bias=shifts[:, 0:1], scale=scales[:, 0:1])
                    nc.scalar.activation(z1, y1[:, c0:c0 + CW], Act.Relu,
                                         bias=shifts[:, 1:2], scale=scales[:, 1:2])
                    fr, fc = divmod(ci, 2)
                    for ot in range(2):
                        o0 = ot * 128
                        cps = cp.tile([128, CW], fp32, tag="cps")
                        nc.tensor.matmul(cps, wt0[:, o0:o0 + 128], z0[:],
                                         start=True, stop=False)
                        nc.tensor.matmul(cps, wt1[:, o0:o0 + 128], z1[:],
                                         start=False, stop=True)
                        osb = zp.tile([128, CW], fp32, tag=f"osb{ot}")
                        nc.vector.scalar_tensor_tensor(
                            out=osb, in0=cps, scalar=cbv[:, ot:ot + 1],
                            in1=xs_t[ot][:, c0:c0 + CW], op0=Alu.add, op1=Alu.add)
                        nc.sync.dma_start(
                            out=out_d[fr, o0:o0 + 128, fc * CW:(fc + 1) * CW],
                            in_=osb)

    nc.finalize()
    return nc


def _get_nc(num_cores):
    if num_cores not in _CACHE:
        _CACHE[num_cores] = _build(num_cores)
    return _CACHE[num_cores]


def _prep_core_inputs(x, conv_w, gamma, beta, conv_b):
    """Build the 8 per-core input dicts from the full problem inputs."""
    xs_all = np.ascontiguousarray(
        x.reshape(4, SNIP, C, F).transpose(0, 2, 1, 3).reshape(4, C, T))
    wt = _round_f32r(np.ascontiguousarray(conv_w.T))
    maps = []
    for k in range(NCORES):
        b, h = divmod(k, 2)
        xs = xs_all[b]
        if h:
            xs = np.ascontiguousarray(
                np.concatenate((xs[:, J:], xs[:, :J]), axis=1))
        nrm = np.sqrt((xs * xs).sum(0, dtype=np.float32))
        xn = xs * (1.0 / nrm)[None, :].astype(np.float32)
        r = _round_f32r(xn)
        xst1 = np.ascontiguousarray(xs.T)
        maps.append({
            "xs": xs,
            "xst": np.ascontiguousarray(np.concatenate([xst1, xst1], axis=0)),
            "xr": r,
            "wt": wt,
            "gam": np.ascontiguousarray(gamma, np.float32),
            "bet": np.ascontiguousarray(beta, np.float32),
            "cb": np.ascontiguousarray(conv_b, np.float32),
        })
    return maps


def kernel(x, gamma, beta, conv_w, conv_b, snip):
    assert int(snip) == SNIP and x.shape == (32, C, 28, 28)
    from concourse.bass_utils import run_bass_kernel_spmd

    x = np.ascontiguousarray(x, np.float32)
    maps = _prep_core_inputs(x, np.asarray(conv_w, np.float32),
                             np.asarray(gamma, np.float32),
                             np.asarray(beta, np.float32),
                             np.asarray(conv_b, np.float32))
    nc = _get_nc(NCORES)
    res = run_bass_kernel_spmd(nc, maps, list(range(NCORES))).results
    out = np.empty((32, C, F), np.float32)
    for k in range(NCORES):
        out[4 * k:4 * k + 4] = res[k]["out"]
    return out.reshape(32, C, 28, 28)


# revision 22
# speedup vs baseline: 3.1386x; 1.0144x over previous
"""Trainium2 Bass kernel for nn_Correspondence (retrieval_knn).

Pipeline per clip (B=4 clips, snip=8 frames of 28x28, C=256):
  xs = [C, THW=6272] per clip; corr = cosine similarity over channels;
  per column j: top-5 rows t (same-frame block excluded) -> gather xs cols,
  max over the 5 -> y; global BatchNorm (training stats) + relu -> 1x1 conv
  -> + identity.

Sharding: 8 cores = 4 clips x 2 column-halves. Each core gets its clip's
xs ROTATED by half the frames so its own j-range is local columns [0,3136)
— the same SPMD program runs on all cores. Same-frame masking is handled
by *never computing* the own-frame columns (frame-rotated chunk order).

Precision: the correlation matmul runs in float32r only (12-bit mantissa,
fp32 accumulate). On the actual seed-0 data this flips the top-5 set on
~43/25088 columns vs exact fp32, giving end-to-end rel err ~5e-3 — well
under the 2e-2 gate. Gather/BN run in exact fp32; the 1x1 conv uses f32r.

Gather strategy: ap_gather costs ~27ns/index (4-idx read requests on the
Q7), which made it the whole-kernel bottleneck. Instead the top-5 columns
are pulled with gpsimd.dma_gather from a host-prepared TRANSPOSED copy
xsT [T, C] in DRAM: each int16 index fetches one 1 KiB row (a full
256-channel column) via SWDGE descriptors at DMA line rate. Index order
i = 16*e + p with e = psi + 8*s, so gather-output row 16*psi+p == j-j0
(identity within a 128-j block) and the 5 candidates of each j land in one
partition at stride 256 — a single strided DVE max-reduce yields yT
[j, c], which two PE transposes turn back into y [c, j] in SBUF.
The wrapped index strip is built from per-tile PE transposes of the fin
indices with 5 strided DMAs per wave. Blocks are wave-pipelined against
the top-k loop. BN stats accumulate per-block on ACT; one AllReduce at
the end, then BN+relu+1x1 conv straight out of SBUF.
"""
import sys, os
import numpy as np

for _p in ("/opt/trn_rl_repo", "/root/.axon_site/_ro/trn_rl_repo"):
    if os.path.isdir(_p) and _p not in sys.path:
        sys.path.insert(0, _p)
        break

# ---------------- problem constants (hardcoded) ----------------
C = 256          # channels
SNIP = 8         # frames per clip
F = 784          # 28*28
T = SNIP * F     # 6272 columns per clip
J = T // 2       # 3136 columns handled per core
JT = 112         # j-tile rows (one PE M-tile; 112*7 = 784 -> tiles never span frames)
NT = J // JT     # 28 j-tiles
TS = 7 * F       # 5488 searched columns per j-tile (own frame excluded)
KTOP = 5
NCORES = 8
NTOT = 32 * F    # batchnorm count = BS*H*W = 25088
CW = 392         # matmul chunk width (one PSUM bank)
NB = 25          # gather blocks of 128 j (last block overlaps, j0=3008)
BW = KTOP * 8    # 40 wrapped-idx elements per block per partition-row

_CACHE = {}


def _round_f32r(x):
    """Round-to-nearest-even to f32r (low 12 mantissa bits zeroed)."""
    b = np.ascontiguousarray(x, np.float32).view(np.uint32)
    low = b & np.uint32(0xFFF)
    add = (low > 0x800) | ((low == 0x800) & (((b >> 12) & 1) == 1))
    b = (b & ~np.uint32(0xFFF)) + (add.astype(np.uint32) << 12)
    return b.view(np.float32)


def _build(num_cores, dbg=False):
    import concourse.bass as bass
    import concourse.mybir as mybir
    import concourse.tile as tile
    from concourse import bacc
    from concourse.masks import make_identity

    fp32 = mybir.dt.float32
    f32r = mybir.dt.float32r
    i16 = mybir.dt.int16
    u16 = mybir.dt.uint16
    Alu = mybir.AluOpType
    Act = mybir.ActivationFunctionType
    Ax = mybir.AxisListType

    nc = bacc.Bacc("TRN2", target_bir_lowering=False, debug=False,
                   num_devices=num_cores)

    xs_d = nc.declare_dram_parameter("xs", [C, T], fp32, isOutput=False)
    # doubled transposed copy: gather indices skip the mod-T wrap entirely
    xst_d = nc.declare_dram_parameter("xst", [2 * T, C], fp32, isOutput=False)
    r_d = nc.declare_dram_parameter("xr", [C, T], f32r, isOutput=False)
    wt_d = nc.declare_dram_parameter("wt", [C, C], f32r, isOutput=False)
    gam_d = nc.declare_dram_parameter("gam", [C], fp32, isOutput=False)
    bet_d = nc.declare_dram_parameter("bet", [C], fp32, isOutput=False)
    cb_d = nc.declare_dram_parameter("cb", [C], fp32, isOutput=False)
    out_d = nc.declare_dram_parameter("out", [4, C, F], fp32, isOutput=True)
    if dbg:
        dbg_fin = nc.declare_dram_parameter("dbg_fin", [JT, 8], fp32, isOutput=True)
        dbg_w16 = nc.declare_dram_parameter("dbg_w16", [128, NB * BW], i16, isOutput=True)
        dbg_gg = nc.declare_dram_parameter("dbg_gg", [128, KTOP, C], fp32, isOutput=True)
        dbg_yt = nc.declare_dram_parameter("dbg_yt", [128, C], fp32, isOutput=True)

    # gather block j-origins: 24 full blocks + overlap block at 3008
    bj0 = [min(128 * b, J - 128) for b in range(NB)]
    # emission plan: after tile `at`, wrapped-idx strip for blocks [b0,b1) is built
    waves = {3: (0, 2), 7: (2, 6), 13: (6, 12), 20: (12, 18),
             25: (18, 22), 26: (22, 23), 27: (23, 25)}

    with tile.TileContext(nc) as tc:
        with tc.tile_pool(name="singles", bufs=1) as sg, \
             tc.tile_pool(name="dram", bufs=1, space="DRAM") as dp:
            # ---- persistent inputs in SBUF
            r0 = sg.tile([128, T], f32r)
            r1 = sg.tile([128, T], f32r)
            xs0 = sg.tile([128, T], fp32)
            xs1 = sg.tile([128, T], fp32)
            y0 = sg.tile([128, J], fp32)
            y1 = sg.tile([128, J], fp32)
            wt0 = sg.tile([128, C], f32r)
            wt1 = sg.tile([128, C], f32r)
            gam = sg.tile([128, 2], fp32)
            bet = sg.tile([128, 2], fp32)
            cbv = sg.tile([128, 2], fp32)
            ident = sg.tile([128, 128], fp32)
            stats = sg.tile([128, 4, NB], fp32)
            astat = sg.tile([128, 4], fp32)
            scales = sg.tile([128, 2], fp32)
            shifts = sg.tile([128, 2], fp32)
            scr = sg.tile([128, 128], fp32)
            trps = sg.tile([8, J], i16)          # transposed top-5 index strip
            w16a = sg.tile([128, NB * BW], i16)  # wrapped idx list, replicated

            cc_in = dp.tile([128, 4], fp32)
            cc_out = dp.tile([128, 4], fp32, addr_space="Shared")

            # correlation operands first: the first j-tile only needs r.
            # xs/wt/BN params are epilogue-only and load later (at jt==1).
            nc.sync.dma_start(out=r0, in_=r_d[0:128, :])
            nc.sync.dma_start(out=r1, in_=r_d[128:256, :])

            make_identity(nc, ident)
            # dummy sqrt: pulls the ACT sqrt table in now, not on the
            # post-allreduce critical path
            nc.scalar.sqrt(scr[:, 0:1], ident[:, 0:1])
            bases = sg.tile([JT, 4], fp32)
            for ff in range(4):
                nc.vector.memset(bases[:, ff:ff + 1], float((ff + 1) * F))

            with tc.tile_pool(name="spool", bufs=2) as sp, \
                 tc.tile_pool(name="work", bufs=2) as wk, \
                 tc.tile_pool(name="gatp", bufs=3) as gp, \
                 tc.tile_pool(name="pp", bufs=4, space="PSUM") as pp, \
                 tc.tile_pool(name="pt", bufs=2, space="PSUM") as pt, \
                 tc.tile_pool(name="py", bufs=2, space="PSUM") as py:

                def emit_w16(b0, b1):
                    """Build wrapped idx list for blocks [b0,b1): w16a[p, b*40+8s+psi]
                    = trps[s, bj0[b] + 8*p + psi], then replicate 16->128.
                    (Gather-out row 16*psi+p then holds j = j0+8p+psi; the
                    block drain un-permutes via strided APs.)"""
                    for b in range(b0, b1):
                        for s in range(KTOP):
                            e0 = b * BW + 8 * s
                            nc.sync.dma_start(
                                out=w16a[0:16, e0:e0 + 8],
                                in_=trps[s:s + 1, bj0[b]:bj0[b] + 128])
                    lo, hi = b0 * BW, b1 * BW
                    nc.sync.dma_start(out=w16a[16:32, lo:hi], in_=w16a[0:16, lo:hi])
                    nc.sync.dma_start(out=w16a[32:64, lo:hi], in_=w16a[0:32, lo:hi])
                    nc.sync.dma_start(out=w16a[64:128, lo:hi], in_=w16a[0:64, lo:hi])

                def emit_gather(b):
                    """Issue the 5-column dma_gather for one 128-j block."""
                    gg = gp.tile([128, KTOP, C], fp32, tag="gg")
                    nc.gpsimd.dma_gather(gg[:], xst_d[:, :],
                                         w16a[:, b * BW:(b + 1) * BW],
                                         KTOP * 128, KTOP * 128, C)
                    return gg

                def emit_reduce(b, gg):
                    """Reduce a gathered block, transpose back to [c, j],
                    accumulate BN stats."""
                    j0b = bj0[b]
                    # pairwise max tree on contiguous [128,C] slices (faster on
                    # DVE than one strided 5-way reduce)
                    m2 = gp.tile([128, 2, C], fp32, tag="m2")
                    yt = gp.tile([128, C], fp32, tag="yt")
                    nc.vector.tensor_tensor(out=m2.rearrange("p a c -> p (a c)"),
                                            in0=gg.rearrange("p s c -> p (s c)")[:, 0:2 * C],
                                            in1=gg.rearrange("p s c -> p (s c)")[:, 2 * C:4 * C],
                                            op=Alu.max)
                    nc.vector.tensor_tensor(out=yt, in0=m2[:, 0, :],
                                            in1=m2[:, 1, :], op=Alu.max)
                    nc.vector.tensor_tensor(out=yt, in0=yt, in1=gg[:, 4, :],
                                            op=Alu.max)
                    ytr = py.tile([128, 2, 128], fp32, tag="ytr")
                    nc.tensor.transpose(ytr[:, 0, :], yt[:, 0:128], ident)
                    nc.tensor.transpose(ytr[:, 1, :], yt[:, 128:256], ident)
                    # gather row r holds j = j0 + 8*(r%16) + r//16 -> un-permute
                    for c, yy in ((0, y0), (1, y1)):
                        dst = yy[:, j0b:j0b + 128].rearrange(
                            "c (p psi) -> c p psi", p=16)
                        src = ytr[:, c, :].rearrange(
                            "c (psi p) -> c p psi", psi=8)
                        nc.scalar.copy(dst, src)
                    # BN partial sums; overlap block only contributes its new cols
                    st0 = 64 if b == NB - 1 else 0
                    for c, yy in ((0, y0), (1, y1)):
                        sl = yy[:, j0b + st0:j0b + 128]
                        nc.scalar.activation(scr[:, st0:128], sl, Act.Identity,
                                             accum_out=stats[:, 2 * c, b:b + 1])
                        nc.scalar.activation(scr[:, st0:128], sl, Act.Square,
                                             accum_out=stats[:, 2 * c + 1, b:b + 1])
                    if dbg and b == 0:
                        nc.sync.dma_start(out=dbg_gg[:], in_=gg)
                        nc.sync.dma_start(out=dbg_yt[:], in_=yt)

                pending = []
                inflight = []
                for jt in range(NT):
                    f = jt // 7                      # local frame of this j-tile
                    j0 = jt * JT
                    if jt == 1:
                        # epilogue-only loads, deferred off the startup path
                        nc.sync.dma_start(out=xs0, in_=xs_d[0:128, :])
                        nc.sync.dma_start(out=xs1, in_=xs_d[128:256, :])
                        nc.sync.dma_start(out=wt0, in_=wt_d[0:128, :])
                        nc.sync.dma_start(out=wt1, in_=wt_d[128:256, :])
                        nc.sync.dma_start(out=gam[:, 0:1], in_=gam_d[0:128])
                        nc.sync.dma_start(out=gam[:, 1:2], in_=gam_d[128:256])
                        nc.sync.dma_start(out=bet[:, 0:1], in_=bet_d[0:128])
                        nc.sync.dma_start(out=bet[:, 1:2], in_=bet_d[128:256])
                        nc.sync.dma_start(out=cbv[:, 0:1], in_=cb_d[0:128])
                        nc.sync.dma_start(out=cbv[:, 1:2], in_=cb_d[128:256])
                    s = sp.tile([JT, TS], fp32, tag="s")

                    # ---- correlation matmuls, frame-rotated column order
                    for ci in range(14):
                        g = (f + 1 + ci // 2) % SNIP  # source frame for chunk
                        gc = g * F + (ci % 2) * CW
                        ps = pp.tile([JT, CW], fp32, tag="ps")
                        nc.tensor.matmul(ps, r0[:, j0:j0 + JT],
                                         r0[:, gc:gc + CW], start=True, stop=False)
                        nc.tensor.matmul(ps, r1[:, j0:j0 + JT],
                                         r1[:, gc:gc + CW], start=False, stop=True)
                        nc.scalar.copy(s[:, ci * CW:(ci + 1) * CW], ps[:])

                    # ---- top-8 values + indices over the full 5488 (exact fp32)
                    t8 = wk.tile([JT, 8], fp32, tag="t8")
                    i8 = wk.tile([JT, 8], u16, tag="i8")
                    nc.vector.max(out=t8, in_=s)
                    nc.vector.max_index(out=i8, in_max=t8, in_values=s)

                    # searched col c -> doubled-xsT row (f+1)*784 + c (no wrap
                    # needed: xst is xs.T twice). Done on ACT to keep DVE free.
                    fin = wk.tile([JT, 8], fp32, tag="fin")
                    nc.scalar.activation(fin, i8, Act.Identity,
                                         bias=bases[:, f:f + 1])
                    if dbg and jt == 0:
                        nc.sync.dma_start(out=dbg_fin[:], in_=fin)

                    # ---- transpose to the index strip (slots on partitions)
                    trp = pt.tile([8, JT], fp32, tag="tr")
                    nc.tensor.transpose(trp, fin, ident[0:JT, 0:JT])
                    nc.scalar.copy(trps[:, j0:j0 + JT], trp)  # fp32 -> i16

                    # ---- wave-pipelined gather blocks: issue gathers ASAP,
                    # defer each reduce one tile so the in-order DVE queue
                    # never stalls on gather data mid-scan
                    if jt in waves:
                        emit_w16(*waves[jt])
                        pending.extend(range(*waves[jt]))
                        if dbg and jt == 27:
                            nc.sync.dma_start(out=dbg_w16[:], in_=w16a)
                    if pending and jt >= 4:
                        b = pending.pop(0)
                        inflight.append((b, emit_gather(b)))
                    if len(inflight) >= 2:
                        emit_reduce(*inflight.pop(0))
                while pending:
                    b = pending.pop(0)
                    inflight.append((b, emit_gather(b)))
                while inflight:
                    emit_reduce(*inflight.pop(0))

            # ---- global batchnorm stats (allreduce over the 8 cores)
            nc.vector.tensor_reduce(out=astat, in_=stats, axis=Ax.X, op=Alu.add)
            nc.sync.dma_start(out=cc_in[:], in_=astat)
            nc.gpsimd.collective_compute(
                "AllReduce", Alu.add,
                replica_groups=[list(range(num_cores))],
                ins=[cc_in[:].opt()], outs=[cc_out[:].opt()])
            nc.sync.dma_start(out=astat, in_=cc_out[:])

            with tc.tile_pool(name="bnw", bufs=1) as bw:
                mean = bw.tile([128, 2], fp32)
                ex2 = bw.tile([128, 2], fp32)
                var = bw.tile([128, 2], fp32)
                std = bw.tile([128, 2], fp32)
                rstd = bw.tile([128, 2], fp32)
                vv = astat.rearrange("p (c m) -> p c m", m=2)
                nc.vector.tensor_scalar_mul(mean, vv[:, :, 0], 1.0 / NTOT)
                nc.vector.tensor_scalar_mul(ex2, vv[:, :, 1], 1.0 / NTOT)
                nc.vector.tensor_tensor(out=var, in0=mean, in1=mean, op=Alu.mult)
                nc.vector.tensor_sub(var, ex2, var)
                nc.vector.tensor_scalar_add(var, var, 1e-5)
                nc.scalar.sqrt(std, var)
                nc.vector.reciprocal(rstd, std)
                nc.vector.tensor_tensor(out=scales, in0=gam, in1=rstd, op=Alu.mult)
                nc.vector.tensor_tensor(out=shifts, in0=mean, in1=scales,
                                        op=Alu.mult)
                nc.vector.tensor_sub(shifts, bet, shifts)

            # ---- BN apply + relu + 1x1 conv + identity + store (y from SBUF)
            xs_t = (xs0, xs1)
            with tc.tile_pool(name="zp", bufs=2) as zp, \
                 tc.tile_pool(name="cp", bufs=2, space="PSUM") as cp:
                for ci in range(8):
                    c0 = ci * CW
                    z0 = zp.tile([128, CW], f32r, tag="z0")
                    z1 = zp.tile([128, CW], f32r, tag="z1")
                    nc.scalar.activation(z0, y0[:, c0:c0 + CW], Act.Relu,
                                         bias=shifts[:, 0:1], scale=scales[:, 0:1])
                    nc.scalar.activation(z1, y1[:, c0:c0 + CW], Act.Relu,
                                         bias=shifts[:, 1:2], scale=scales[:, 1:2])
                    fr, fc = divmod(ci, 2)
                    for ot in range(2):
                        o0 = ot * 128
                        cps = cp.tile([128, CW], fp32, tag="cps")
                        nc.tensor.matmul(cps, wt0[:, o0:o0 + 128], z0[:],
                                         start=True, stop=False)
                        nc.tensor.matmul(cps, wt1[:, o0:o0 + 128], z1[:],
                                         start=False, stop=True)
                        osb = zp.tile([128, CW], fp32, tag=f"osb{ot}")
                        nc.vector.scalar_tensor_tensor(
                            out=osb, in0=cps, scalar=cbv[:, ot:ot + 1],
                            in1=xs_t[ot][:, c0:c0 + CW], op0=Alu.add, op1=Alu.add)
                        nc.sync.dma_start(
                            out=out_d[fr, o0:o0 + 128, fc * CW:(fc + 1) * CW],
                            in_=osb)

    nc.finalize()
    return nc


def _get_nc(num_cores):
    if num_cores not in _CACHE:
        _CACHE[num_cores] = _build(num_cores)
    return _CACHE[num_cores]


def _prep_core_inputs(x, conv_w, gamma, beta, conv_b):
    """Build the 8 per-core input dicts from the full problem inputs."""
    xs_all = np.ascontiguousarray(
        x.reshape(4, SNIP, C, F).transpose(0, 2, 1, 3).reshape(4, C, T))
    wt = _round_f32r(np.ascontiguousarray(conv_w.T))
    maps = []
    for k in range(NCORES):
        b, h = divmod(k, 2)
        xs = xs_all[b]
        if h:
            xs = np.ascontiguousarray(
                np.concatenate((xs[:, J:], xs[:, :J]), axis=1))
        nrm = np.sqrt((xs * xs).sum(0, dtype=np.float32))
        xn = xs * (1.0 / nrm)[None, :].astype(np.float32)
        r = _round_f32r(xn)
        xst1 = np.ascontiguousarray(xs.T)
        maps.append({
            "xs": xs,
            "xst": np.ascontiguousarray(np.concatenate([xst1, xst1], axis=0)),
            "xr": r,
            "wt": wt,
            "gam": np.ascontiguousarray(gamma, np.float32),
            "bet": np.ascontiguousarray(beta, np.float32),
            "cb": np.ascontiguousarray(conv_b, np.float32),
        })
    return maps


def kernel(x, gamma, beta, conv_w, conv_b, snip):
    assert int(snip) == SNIP and x.shape == (32, C, 28, 28)
    from concourse.bass_utils import run_bass_kernel_spmd

    x = np.ascontiguousarray(x, np.float32)
    maps = _prep_core_inputs(x, np.asarray(conv_w, np.float32),
                             np.asarray(gamma, np.float32),
                             np.asarray(beta, np.float32),
                             np.asarray(conv_b, np.float32))
    nc = _get_nc(NCORES)
    res = run_bass_kernel_spmd(nc, maps, list(range(NCORES))).results
    out = np.empty((32, C, F), np.float32)
    for k in range(NCORES):
        out[4 * k:4 * k + 4] = res[k]["out"]
    return out.reshape(32, C, 28, 28)
